# revision 19
# baseline (speedup 1.0000x reference)
"""Trainium2 Bass kernel for nn_Attention_8495445311883.

Encoder (bi-RNN) + decoder + dot-attention + output projection.
Sharding: data-parallel over batch B=32 across 8 NeuronCores (4 batches/core).
All matmuls fp16 (fp32 PSUM accumulate). Host pre-packs/transposes weights.

Per-core column index c = b_local*T + t  (b-major), C = 4*T = 512.

v2 schedule (psum-resident pre-activations, LDWEIGHTS-aware):
  A-f   : enc fwd input proj -> PSUM banks 0-3 (PF), kept alive
  f-scan: per step, 16 recurrence mms accumulate Whh_f·h into PF[:, :, t::T]
          (start=False on top of the pre-activations), tanh reads PSUM
          directly; enc BWD input proj (A-b -> PB banks 4-7) interleaves
          as PE filler
  b-scan: same on PB; pass-B (dec input proj) interleaves as filler,
          accumulating into the freed PF banks
  mix   : h0/q chain, ENC proj, DEC tanh straight from PF psum, ENCT
  attn  : software-pipelined over the 4 local batches
  final : 20 V-chunks of 500, fp16 output DMA
"""
import os
import sys
import numpy as np

sys.path.insert(0, "/opt/trn_rl_repo")

V, H, T, B = 10000, 512, 128, 32
NCORES = 8
BL = B // NCORES            # 4 local batches
C = BL * T                  # 512 columns per core
VP = 10112                  # V padded to 79*128
KV = VP // 128              # 79 contraction tiles
NV, VC = 20, 500            # output V chunks: 20 x 500
KG = 4                      # k-tiles per DMA chunk
NKG = (KV + KG - 1) // KG   # 20 chunks, last has 3

WHH_FP8 = bool(int(os.environ.get("WHH_FP8", "0")))

_cached = {}


def _build_nc(reps=1, phases='ASBMF'):
    import concourse.bacc as bacc
    import concourse.mybir as mybir
    import concourse.tile as tile

    dt = mybir.dt
    AF = mybir.ActivationFunctionType
    AX = mybir.AxisListType
    whh_dt = dt.float8e4 if WHH_FP8 else dt.float16

    nc = bacc.Bacc(None, target_bir_lowering=False)

    xT = nc.dram_tensor("xT", [VP, C], dt.float16, kind="ExternalInput")
    dxT = nc.dram_tensor("dxT", [VP, C], dt.float16, kind="ExternalInput")
    WIH = nc.dram_tensor("WIH", [VP, 3 * H], dt.float16, kind="ExternalInput")
    WO = nc.dram_tensor("WO", [2 * H, V], dt.float16, kind="ExternalInput")
    WHH = nc.dram_tensor("WHH", [H, 3 * H], whh_dt, kind="ExternalInput")
    A1 = nc.dram_tensor("A1", [2 * H, H], dt.float16, kind="ExternalInput")
    A2 = nc.dram_tensor("A2", [2 * H, H], dt.float16, kind="ExternalInput")
    CONST = nc.dram_tensor("CONST", [128, 12], dt.float32, kind="ExternalInput")
    BOUT = nc.dram_tensor("BOUT", [1, V], dt.float16, kind="ExternalInput")
    ONES = nc.dram_tensor("ONES", [1, 128], dt.float16, kind="ExternalInput")
    IDN16 = nc.dram_tensor("IDN16", [128, 128], dt.float16, kind="ExternalInput")
    ENCH = nc.dram_tensor("ENCH", [128, 32], whh_dt, kind="ExternalInput")
    out = nc.dram_tensor("out", [BL, T, V], dt.float16, kind="ExternalOutput")

    xTr = xT.rearrange("(k p) c -> p k c", p=128)
    dxTr = dxT.rearrange("(k p) c -> p k c", p=128)
    WIr = WIH.rearrange("(k p) c -> p k c", p=128)
    WOr = WO.rearrange("(k p) v -> p k v", p=128)
    outr = out.rearrange("b t v -> t b v")

    with tile.TileContext(nc) as tc:
        with (
            tc.tile_pool(name="const", bufs=1) as cp,
            tc.tile_pool(name="acts", bufs=1) as ap,
            tc.tile_pool(name="xs", bufs=3) as xs,
            tc.tile_pool(name="ws", bufs=3) as ws,
            tc.tile_pool(name="os", bufs=4) as osp,
        ):
            # ---- persistent activations ----
            OUTF = ap.tile([128, 4, C], dt.float16, tag="OUTF")
            OUTB = ap.tile([128, 4, C], dt.float16, tag="OUTB")
            ENC = ap.tile([128, 4, C], dt.float16, tag="ENC")
            ENCT = ap.tile([128, 4, C], dt.float16, tag="ENCT")
            DEC = ap.tile([128, 4, C], dt.float16, tag="DEC")
            CTX = ap.tile([128, 4, C], dt.float16, tag="CTX")
            H0 = ap.tile([128, 4, 4], dt.float16, tag="H0")
            Q = ap.tile([128, 4, 4], dt.float32, tag="Q")
            if WHH_FP8:
                OUTF8 = ap.tile([128, 4, C], dt.float8e4, tag="OUTF8")
                OUTB8 = ap.tile([128, 4, C], dt.float8e4, tag="OUTB8")
            else:
                OUTF8, OUTB8 = OUTF, OUTB

            # ---- resident constants/weights (ACT queue; after first chunks) ----
            def load_consts():
                whh = cp.tile([128, 4, 3 * H], whh_dt, tag="whh")
                nc.scalar.dma_start(whh[:], WHH.rearrange("(j p) c -> p j c", p=128))
                a1 = cp.tile([128, 8, H], dt.float16, tag="a1")
                nc.scalar.dma_start(a1[:], A1.rearrange("(j p) c -> p j c", p=128))
                a2 = cp.tile([128, 8, H], dt.float16, tag="a2")
                nc.scalar.dma_start(a2[:], A2.rearrange("(j p) c -> p j c", p=128))
                cst = cp.tile([128, 12], dt.float32, tag="cst")
                nc.scalar.dma_start(cst[:], CONST[:])
                bout = cp.tile([1, V], dt.float16, tag="bout")
                nc.scalar.dma_start(bout[:], BOUT[:])
                ones = cp.tile([1, 128], dt.float16, tag="ones")
                nc.scalar.dma_start(ones[:], ONES[:])
                idn16 = cp.tile([128, 128], dt.float16, tag="idn16")
                nc.scalar.dma_start(idn16[:], IDN16[:])
                ench = cp.tile([128, 32], whh_dt, tag="ench")
                nc.scalar.dma_start(ench[:], ENCH[:])
                return whh, a1, a2, cst, bout, ones, idn16, ench

            consts = None

            for _rep in range(reps):
                # ========== pass A-f: enc fwd input proj into PF ==========
                p1_cm = tc.tile_pool(name="p1", bufs=1, space="PSUM")
                p1 = p1_cm.__enter__()
                PF = p1.tile([128, 4, C], dt.float32, tag="PF", name="PF")
                p2_cm = tc.tile_pool(name="p2", bufs=1, space="PSUM")
                p2 = p2_cm.__enter__()
                PB = p2.tile([128, 4, C], dt.float32, tag="PB", name="PB")

                af_tiles = []
                for g in range(NKG):
                    ks = (g * KG, min((g + 1) * KG, KV))
                    nk = ks[1] - ks[0]
                    xk = xs.tile([128, KG, C], dt.float16, tag="xk")
                    nc.sync.dma_start(xk[:, :nk, :], xTr[:, ks[0]:ks[1], :])
                    wk = ws.tile([128, KG, H], dt.float16, tag="wk")
                    nc.scalar.dma_start(wk[:, :nk, :], WIr[:, ks[0]:ks[1], 0:H])
                    af_tiles.append((xk, wk, ks))
                    if g == 5 and consts is None:
                        consts = load_consts()
                whh, a1, a2, cst, bout, ones, idn16, ench = consts

                for xk, wk, ks in (af_tiles if 'A' in phases else []):
                    for i in range(ks[1] - ks[0]):
                        k = ks[0] + i
                        for m in range(4):
                            nc.tensor.matmul(
                                PF[:, m, :], wk[:, i, m * 128:(m + 1) * 128],
                                xk[:, i, :],
                                start=(k == 0), stop=(k == KV - 1),
                            )

                # ========== f-scan (A-b as PE filler into PB) ==========
                # A-b chunk DMAs: re-read xT, bwd weight columns
                ab_tiles = []
                for g in range(NKG):
                    ks = (g * KG, min((g + 1) * KG, KV))
                    nk = ks[1] - ks[0]
                    xk = xs.tile([128, KG, C], dt.float16, tag="xk")
                    nc.sync.dma_start(xk[:, :nk, :], xTr[:, ks[0]:ks[1], :])
                    wk = ws.tile([128, KG, H], dt.float16, tag="wk")
                    nc.scalar.dma_start(wk[:, :nk, :], WIr[:, ks[0]:ks[1], H:2 * H])
                    ab_tiles.append((xk, wk, ks))

                def filler_gen(tiles, dst):
                    for xk, wk, ks in tiles:
                        for i in range(ks[1] - ks[0]):
                            k = ks[0] + i
                            for m in range(4):
                                nc.tensor.matmul(
                                    dst[:, m, :],
                                    wk[:, i, m * 128:(m + 1) * 128],
                                    xk[:, i, :],
                                    start=(k == 0), stop=(k == KV - 1),
                                )
                                yield

                # PB column c=b*T+u holds pre_b for input time u; the b-scan
                # walks tb = T-1-t downward, so no host-side reversal needed.
                abgen = filler_gen(ab_tiles, PB)
                abdone = [0]
                TOTF = KV * 4

                def pump_ab(target):
                    while abdone[0] < min(target, TOTF):
                        try:
                            next(abgen)
                        except StopIteration:
                            abdone[0] = TOTF
                            return
                        abdone[0] += 1

                # columns are t-major per core: c = t*BL + b, so each scan
                # step's psum slice is contiguous (strided psum matmul
                # outputs are unsupported)
                if 'S' in phases:
                    for t in range(T):
                        sl = slice(t * BL, (t + 1) * BL)
                        slp = slice((t - 1) * BL, t * BL)
                        for m in range(4):
                            for j in range(4):
                                rf = ench[:, j * 4:(j + 1) * 4] if t == 0 else \
                                    OUTF8[:, j, slp]
                                nc.tensor.matmul(
                                    PF[:, m, sl],
                                    whh[:, j, m * 128:(m + 1) * 128], rf,
                                    start=False, stop=(j == 3),
                                    skip_group_check=True,
                                )
                        nc.scalar.activation(OUTF[:, :, sl], PF[:, :, sl],
                                             AF.Tanh)
                        if WHH_FP8:
                            nc.vector.tensor_copy(OUTF8[:, :, sl],
                                                  OUTF[:, :, sl])
                        pump_ab((t + 1) * TOTF // 110 + 1)
                if 'B' in phases or 'S' in phases:
                    pump_ab(TOTF)

                # ========== b-scan (pass B as PE filler into PF) ==========
                b_tiles = []
                for g in range(NKG):
                    ks = (g * KG, min((g + 1) * KG, KV))
                    nk = ks[1] - ks[0]
                    dk = xs.tile([128, KG, C], dt.float16, tag="xk")
                    nc.sync.dma_start(dk[:, :nk, :], dxTr[:, ks[0]:ks[1], :])
                    wkd = ws.tile([128, KG, H], dt.float16, tag="wk")
                    nc.scalar.dma_start(wkd[:, :nk, :], WIr[:, ks[0]:ks[1], 2 * H:])
                    b_tiles.append((dk, wkd, ks))
                # final-proj weight chunks: prefetch on the ACT queue
                won_tiles = {}

                def won_dma(n):
                    won = ws.tile([128, 8, VC], dt.float16, tag="won")
                    nc.scalar.dma_start(won[:], WOr[:, :, n * VC:(n + 1) * VC])
                    won_tiles[n] = won

                if 'F' in phases:
                    won_dma(0)
                    won_dma(1)

                bgen = filler_gen(b_tiles, PF)
                bdone = [0]

                def pump_b(target):
                    while bdone[0] < min(target, TOTF):
                        try:
                            next(bgen)
                        except StopIteration:
                            bdone[0] = TOTF
                            return
                        bdone[0] += 1

                if 'S' in phases:
                    for t in range(T):
                        tb = T - 1 - t
                        sl = slice(tb * BL, (tb + 1) * BL)
                        slp = slice((tb + 1) * BL, (tb + 2) * BL)
                        for m in range(4):
                            for j in range(4):
                                rb = ench[:, 16 + j * 4:16 + (j + 1) * 4] \
                                    if t == 0 else OUTB8[:, j, slp]
                                nc.tensor.matmul(
                                    PB[:, m, sl],
                                    whh[:, j, H + m * 128:H + (m + 1) * 128], rb,
                                    start=False, stop=(j == 3),
                                    skip_group_check=True,
                                )
                        nc.scalar.activation(OUTB[:, :, sl], PB[:, :, sl],
                                             AF.Tanh)
                        if WHH_FP8:
                            nc.vector.tensor_copy(OUTB8[:, :, sl],
                                                  OUTB[:, :, sl])
                        pump_b((t + 1) * TOTF // 110 + 1)
                if 'B' in phases or 'S' in phases:
                    pump_b(TOTF)

                # ========== mix phase ==========
                # PB's pre_b is fully consumed; reuse two of its column
                # regions as the tiny h0/q psum so no new pool is needed
                # while PF (pass-B result) is still alive.

                # h0 = A1 @ [h_f; h_b] + b_attn1
                for m in (range(4) if 'M' in phases else []):
                    for k in range(8):
                        rh = OUTF[:, k, (T - 1) * BL:T * BL] if k < 4 \
                            else OUTB[:, k - 4, 0:BL]
                        nc.tensor.matmul(PB[:, 0, m * 4:(m + 1) * 4],
                                         a1[:, k, m * 128:(m + 1) * 128],
                                         rh, start=(k == 0), stop=(k == 7),
                                         skip_group_check=True)
                for m in (range(4) if 'M' in phases else []):
                    nc.scalar.activation(H0[:, m, :], PB[:, 0, m * 4:(m + 1) * 4],
                                         AF.Identity, bias=cst[:, m:m + 1])
                # q = Whh_d @ h0 + bhh_d   (whh fp8 x H0 fp16 mixed when WHH_FP8;
                # cast H0 to whh dtype via DVE if needed)
                if WHH_FP8:
                    H0q = osp.tile([128, 4, 4], dt.float8e4, tag="h0q")
                    if 'M' in phases:
                        nc.vector.tensor_copy(H0q[:], H0[:])
                else:
                    H0q = H0
                # q = Whh_d @ h0 + bhh_d, psum in PB region 1
                for m in (range(4) if 'M' in phases else []):
                    for j in range(4):
                        nc.tensor.matmul(
                            PB[:, 1, m * 4:(m + 1) * 4],
                            whh[:, j, 2 * H + m * 128:2 * H + (m + 1) * 128],
                            H0q[:, j, :], start=(j == 0), stop=(j == 3),
                            skip_group_check=True,
                        )
                for m in (range(4) if 'M' in phases else []):
                    nc.scalar.activation(Q[:, m, :], PB[:, 1, m * 4:(m + 1) * 4],
                                         AF.Identity, bias=cst[:, 8 + m:9 + m])

                # DEC tanh straight from PF (pass-B) psum; psum cols are
                # t-major so batch b is the strided read b::BL, while DEC
                # itself is written b-major for contiguous matmul stationaries
                for m in (range(4) if 'M' in phases else []):
                    for b in range(BL):
                        nc.scalar.activation(
                            DEC[:, m, b * T:(b + 1) * T], PF[:, m, b::BL],
                            AF.Tanh, bias=Q[:, m, b:b + 1],
                        )

                # PB then PF dead -> LIFO pool exits, then one mix pool
                p2_cm.__exit__(None, None, None)
                p1_cm.__exit__(None, None, None)
                pmix_cm = tc.tile_pool(name="pmix", bufs=1, space="PSUM")
                pmix = pmix_cm.__enter__()

                # ENC = W_attn2 @ [out_f; out_b] + b_attn2, ENCT transposes
                # pipelined on PE behind each m's ACT
                def enc_mms(m):
                    pe = pmix.tile([128, C], dt.float32, tag="pe2", bufs=2)
                    for k in range(8):
                        src = OUTF if k < 4 else OUTB
                        nc.tensor.matmul(
                            pe[:], a2[:, k, m * 128:(m + 1) * 128], src[:, k % 4, :],
                            start=(k == 0), stop=(k == 7),
                        )
                    return pe

                def enct_tr(m):
                    for b in range(BL):
                        ptr = pmix.tile([128, 128], dt.float16, tag="ptb", bufs=2)
                        nc.tensor.transpose(ptr[:], ENC[:, m, b * T:(b + 1) * T],
                                            idn16[:])
                        nc.vector.tensor_copy(
                            ENCT[:, b, m * 128:(m + 1) * 128], ptr[:])

                pe_tiles = {}
                for m in (range(4) if 'M' in phases else []):
                    pe_tiles[m] = enc_mms(m)
                    # psum cols t-major -> ENC written b-major via AP permute
                    nc.scalar.activation(
                        ENC[:, m, :].rearrange("p (b t) -> p b t", b=BL),
                        pe_tiles[m][:].rearrange("p (t b) -> p b t", b=BL),
                        AF.Identity, bias=cst[:, 4 + m:5 + m])
                    if m > 0:
                        enct_tr(m - 1)
                if 'M' in phases:
                    enct_tr(3)

                # ---- attention, software-pipelined over batches ----
                def scores_chain(b):
                    ps = pmix.tile([128, 128], dt.float32, tag="pf128", bufs=2)
                    for k in range(4):
                        nc.tensor.matmul(
                            ps[:], DEC[:, k, b * T:(b + 1) * T],
                            ENC[:, k, b * T:(b + 1) * T],
                            start=(k == 0), stop=(k == 3),
                        )
                    negm = osp.tile([128, 1], dt.float32, tag="negm")
                    nc.vector.reduce_max(negm[:], ps[:], axis=AX.X, negate=True)
                    prob = osp.tile([128, T], dt.float16, tag="prob")
                    rsum = osp.tile([128, 1], dt.float32, tag="rsum")
                    nc.scalar.activation(prob[:], ps[:], AF.Exp, bias=negm[:],
                                         accum_out=rsum[:])
                    rinv = osp.tile([128, 1], dt.float32, tag="rinv")
                    nc.vector.reciprocal(rinv[:], rsum[:])
                    nc.vector.tensor_scalar_mul(prob[:], prob[:], rinv[:])
                    return prob

                def ctx_chain(b, prob):
                    pwt = pmix.tile([128, 128], dt.float16, tag="ptb", bufs=2)
                    nc.tensor.transpose(pwt[:], prob[:], idn16[:])
                    wt = osp.tile([128, T], dt.float16, tag="wt")
                    nc.vector.tensor_copy(wt[:], pwt[:])
                    for m in range(4):
                        pc = pmix.tile([128, 128], dt.float32, tag="pf128", bufs=2)
                        nc.tensor.matmul(pc[:], ENCT[:, b, m * 128:(m + 1) * 128],
                                         wt[:], start=True, stop=True)
                        nc.vector.tensor_copy(CTX[:, m, b * T:(b + 1) * T], pc[:])

                probs = {}
                for b in (range(BL) if 'M' in phases else []):
                    probs[b] = scores_chain(b)
                    if b > 0:
                        ctx_chain(b - 1, probs[b - 1])
                if 'M' in phases:
                    ctx_chain(3, probs[3])
                pmix_cm.__exit__(None, None, None)

                # ========== final projection: predict[c, v] ==========
                pf_cm = tc.tile_pool(name="pf", bufs=1, space="PSUM")
                pf = pf_cm.__enter__()
                for n in (range(NV) if 'F' in phases else []):
                    won = won_tiles[n]
                    pbias = pf.tile([128, VC], dt.float32, tag="pbias", bufs=2)
                    nc.tensor.matmul(pbias[:], ones[0:1, :],
                                     bout[0:1, n * VC:(n + 1) * VC],
                                     start=True, stop=True)
                    bias_sb = osp.tile([128, VC], dt.float16, tag="bsb", bufs=2)
                    nc.scalar.activation(bias_sb[:], pbias[:], AF.Copy)
                    if n + 2 < NV:
                        won_dma(n + 2)
                    ob = osp.tile([128, BL, VC], dt.float16, tag="ob", bufs=2)
                    for b in range(BL):
                        po = pf.tile([128, VC], dt.float32, tag="po", bufs=4)
                        for k in range(8):
                            src = CTX if k < 4 else DEC
                            nc.tensor.matmul(
                                po[:], src[:, k % 4, b * T:(b + 1) * T],
                                won[:, k, :],
                                start=(k == 0), stop=(k == 7),
                            )
                        nc.vector.tensor_add(ob[:, b, :], po[:], bias_sb[:])
                    nc.sync.dma_start(outr[:, :, n * VC:(n + 1) * VC], ob[:])
                pf_cm.__exit__(None, None, None)

    nc.compile()
    return nc


def _pack(inputs):
    """Host-side packing: shared weights + per-core activation shards."""
    import ml_dtypes
    f = {k: np.asarray(v, dtype=np.float32) for k, v in inputs.items()}

    def bf(a):
        return np.ascontiguousarray(a.astype(np.float16))

    def q8(a):
        return np.ascontiguousarray(a.astype(ml_dtypes.float8_e4m3))

    qw = q8 if WHH_FP8 else bf

    WIH = np.zeros((VP, 3 * H), np.float32)
    WIH[:V, 0:H] = f["Wih_f"].T
    WIH[:V, H:2 * H] = f["Wih_b"].T
    WIH[:V, 2 * H:] = f["Wih_d"].T
    WIH[V, 0:H] = f["bih_f"] + f["bhh_f"]
    WIH[V, H:2 * H] = f["bih_b"] + f["bhh_b"]
    WIH[V, 2 * H:] = f["bih_d"]

    WHH = np.concatenate([f["Whh_f"].T, f["Whh_b"].T, f["Whh_d"].T], axis=1)
    CONST = np.concatenate(
        [f["b_attn1"].reshape(4, 128).T, f["b_attn2"].reshape(4, 128).T,
         f["bhh_d"].reshape(4, 128).T], axis=1).astype(np.float32)

    shared = {
        "WIH": bf(WIH),
        "WO": bf(f["W_out"].T),
        "WHH": qw(WHH),
        "A1": bf(f["W_attn1"].T),
        "A2": bf(f["W_attn2"].T),
        "CONST": np.ascontiguousarray(CONST),
        "BOUT": bf(f["b_out"].reshape(1, V)),
        "ONES": bf(np.ones((1, 128), np.float32)),
        "IDN16": np.ascontiguousarray(np.eye(128, dtype=np.float16)),
    }

    # activations: [VP, T, B]; per-core column c = t*BL + b (t-major), pad
    # to VP with ones row at V
    def actT(a):  # [B, T, V] -> [VP, T, B]
        r = np.zeros((VP, T, B), np.float32)
        r[:V] = a.transpose(2, 1, 0)
        r[V] = 1.0
        return bf(r)

    XT = actT(f["enc_inputs"])
    DXT = actT(f["dec_inputs"])

    in_maps = []
    for core in range(NCORES):
        sl = slice(core * BL, (core + 1) * BL)
        eh = np.zeros((128, 32), np.float32)
        for d in range(2):
            hh = f["enc_hidden"][d, core * BL:(core + 1) * BL]     # [4, 512]
            eh[:, d * 16:(d + 1) * 16] = \
                hh.T.reshape(4, 128, 4).transpose(1, 0, 2).reshape(128, 16)
        m = dict(shared)
        m["xT"] = np.ascontiguousarray(XT[:, :, sl]).reshape(VP, C)
        m["dxT"] = np.ascontiguousarray(DXT[:, :, sl]).reshape(VP, C)
        m["ENCH"] = qw(eh)
        in_maps.append(m)
    return in_maps


def kernel(**inputs):
    from concourse.bass_utils import run_bass_kernel_spmd

    if "nc" not in _cached:
        _cached["nc"] = _build_nc()
    nc = _cached["nc"]
    in_maps = _pack(inputs)
    res = run_bass_kernel_spmd(
        nc, in_maps, core_ids=list(range(NCORES)),
        trace=bool(int(os.environ.get("KTRACE", "0"))),
    )
    _cached["last"] = res
    outp = np.zeros((B, T, V), np.float32)
    for core in range(NCORES):
        outp[core * BL:(core + 1) * BL] = \
            np.asarray(res.results[core]["out"], np.float32)
    return outp


# revision 21
# speedup vs baseline: 1.5366x; 1.5366x over previous
"""Trainium2 Bass kernel for nn_Attention_8495445311883.

Encoder (bi-RNN) + decoder + dot-attention + output projection.
Sharding: data-parallel over batch B=32 across 8 NeuronCores (4 batches/core).
All matmuls fp16 (fp32 PSUM accumulate). Host pre-packs/transposes weights.

Per-core column index c = b_local*T + t  (b-major), C = 4*T = 512.

v2 schedule (psum-resident pre-activations, LDWEIGHTS-aware):
  A-f   : enc fwd input proj -> PSUM banks 0-3 (PF), kept alive
  f-scan: per step, 16 recurrence mms accumulate Whh_f·h into PF[:, :, t::T]
          (start=False on top of the pre-activations), tanh reads PSUM
          directly; enc BWD input proj (A-b -> PB banks 4-7) interleaves
          as PE filler
  b-scan: same on PB; pass-B (dec input proj) interleaves as filler,
          accumulating into the freed PF banks
  mix   : h0/q chain, ENC proj, DEC tanh straight from PF psum, ENCT
  attn  : software-pipelined over the 4 local batches
  final : 20 V-chunks of 500, fp16 output DMA
"""
import os
import sys
import numpy as np

sys.path.insert(0, "/opt/trn_rl_repo")

V, H, T, B = 10000, 512, 128, 32
NCORES = 8
BL = B // NCORES            # 4 local batches
C = BL * T                  # 512 columns per core
VP = 10112                  # V padded to 79*128
KV = VP // 128              # 79 contraction tiles
NV, VC = 20, 500            # output V chunks: 20 x 500
KG = 4                      # k-tiles per DMA chunk
NKG = (KV + KG - 1) // KG   # 20 chunks, last has 3

WHH_FP8 = bool(int(os.environ.get("WHH_FP8", "0")))

_cached = {}


def _build_nc(reps=1, phases='ASBMF'):
    import concourse.bacc as bacc
    import concourse.mybir as mybir
    import concourse.tile as tile

    dt = mybir.dt
    AF = mybir.ActivationFunctionType
    AX = mybir.AxisListType
    whh_dt = dt.float8e4 if WHH_FP8 else dt.float16

    nc = bacc.Bacc(None, target_bir_lowering=False)

    xT = nc.dram_tensor("xT", [VP, C], dt.float16, kind="ExternalInput")
    dxT = nc.dram_tensor("dxT", [VP, C], dt.float16, kind="ExternalInput")
    WIH = nc.dram_tensor("WIH", [VP, 3 * H], dt.float16, kind="ExternalInput")
    WO = nc.dram_tensor("WO", [2 * H, V], dt.float16, kind="ExternalInput")
    WHH = nc.dram_tensor("WHH", [H, 3 * H], whh_dt, kind="ExternalInput")
    A1 = nc.dram_tensor("A1", [2 * H, H], dt.float16, kind="ExternalInput")
    A2 = nc.dram_tensor("A2", [2 * H, H], dt.float16, kind="ExternalInput")
    CONST = nc.dram_tensor("CONST", [128, 12], dt.float32, kind="ExternalInput")
    BOUT = nc.dram_tensor("BOUT", [1, V], dt.float16, kind="ExternalInput")
    ONES = nc.dram_tensor("ONES", [1, 128], dt.float16, kind="ExternalInput")
    IDN16 = nc.dram_tensor("IDN16", [128, 128], dt.float16, kind="ExternalInput")
    ENCH = nc.dram_tensor("ENCH", [128, 32], whh_dt, kind="ExternalInput")
    out = nc.dram_tensor("out", [BL, T, V], dt.float16, kind="ExternalOutput")

    xTr = xT.rearrange("(k p) c -> p k c", p=128)
    dxTr = dxT.rearrange("(k p) c -> p k c", p=128)
    WIr = WIH.rearrange("(k p) c -> p k c", p=128)
    WOr = WO.rearrange("(k p) v -> p k v", p=128)
    outr = out.rearrange("b t v -> t b v")

    with tile.TileContext(nc) as tc:
        with (
            tc.tile_pool(name="const", bufs=1) as cp,
            tc.tile_pool(name="acts", bufs=1) as ap,
            tc.tile_pool(name="xs", bufs=3) as xs,
            tc.tile_pool(name="ws", bufs=3) as ws,
            tc.tile_pool(name="os", bufs=4) as osp,
        ):
            # ---- persistent activations ----
            OUTF = ap.tile([128, 4, C], dt.float16, tag="OUTF")
            OUTB = ap.tile([128, 4, C], dt.float16, tag="OUTB")
            ENC = ap.tile([128, 4, C], dt.float16, tag="ENC")
            ENCT = ap.tile([128, 4, C], dt.float16, tag="ENCT")
            DEC = ap.tile([128, 4, C], dt.float16, tag="DEC")
            CTX = ap.tile([128, 4, C], dt.float16, tag="CTX")
            H0 = ap.tile([128, 4, 4], dt.float16, tag="H0")
            Q = ap.tile([128, 4, 4], dt.float32, tag="Q")
            if WHH_FP8:
                OUTF8 = ap.tile([128, 4, C], dt.float8e4, tag="OUTF8")
                OUTB8 = ap.tile([128, 4, C], dt.float8e4, tag="OUTB8")
            else:
                OUTF8, OUTB8 = OUTF, OUTB

            # ---- resident constants/weights (ACT queue; after first chunks) ----
            def load_consts():
                whh = cp.tile([128, 4, 3 * H], whh_dt, tag="whh")
                nc.scalar.dma_start(whh[:], WHH.rearrange("(j p) c -> p j c", p=128))
                a1 = cp.tile([128, 8, H], dt.float16, tag="a1")
                nc.scalar.dma_start(a1[:], A1.rearrange("(j p) c -> p j c", p=128))
                a2 = cp.tile([128, 8, H], dt.float16, tag="a2")
                nc.scalar.dma_start(a2[:], A2.rearrange("(j p) c -> p j c", p=128))
                cst = cp.tile([128, 12], dt.float32, tag="cst")
                nc.scalar.dma_start(cst[:], CONST[:])
                bout = cp.tile([1, V], dt.float16, tag="bout")
                nc.scalar.dma_start(bout[:], BOUT[:])
                ones = cp.tile([1, 128], dt.float16, tag="ones")
                nc.scalar.dma_start(ones[:], ONES[:])
                idn16 = cp.tile([128, 128], dt.float16, tag="idn16")
                nc.scalar.dma_start(idn16[:], IDN16[:])
                ench = cp.tile([128, 32], whh_dt, tag="ench")
                nc.scalar.dma_start(ench[:], ENCH[:])
                return whh, a1, a2, cst, bout, ones, idn16, ench

            consts = None

            for _rep in range(reps):
                # ========== pass A-f: enc fwd input proj into PF ==========
                p1_cm = tc.tile_pool(name="p1", bufs=1, space="PSUM")
                p1 = p1_cm.__enter__()
                PF = p1.tile([128, 4, C], dt.float32, tag="PF", name="PF")
                p2_cm = tc.tile_pool(name="p2", bufs=1, space="PSUM")
                p2 = p2_cm.__enter__()
                PB = p2.tile([128, 4, C], dt.float32, tag="PB", name="PB")

                af_tiles = []
                for g in range(NKG):
                    ks = (g * KG, min((g + 1) * KG, KV))
                    nk = ks[1] - ks[0]
                    xk = xs.tile([128, KG, C], dt.float16, tag="xk")
                    nc.sync.dma_start(xk[:, :nk, :], xTr[:, ks[0]:ks[1], :])
                    wk = ws.tile([128, KG, H], dt.float16, tag="wk")
                    nc.scalar.dma_start(wk[:, :nk, :], WIr[:, ks[0]:ks[1], 0:H])
                    af_tiles.append((xk, wk, ks))
                    if g == 5 and consts is None:
                        consts = load_consts()
                whh, a1, a2, cst, bout, ones, idn16, ench = consts

                for xk, wk, ks in (af_tiles if 'A' in phases else []):
                    for i in range(ks[1] - ks[0]):
                        k = ks[0] + i
                        for m in range(4):
                            nc.tensor.matmul(
                                PF[:, m, :], wk[:, i, m * 128:(m + 1) * 128],
                                xk[:, i, :],
                                start=(k == 0), stop=(k == KV - 1),
                            )

                # ========== f-scan (A-b as PE filler into PB) ==========
                # A-b chunk DMAs: re-read xT, bwd weight columns
                ab_tiles = []
                for g in range(NKG):
                    ks = (g * KG, min((g + 1) * KG, KV))
                    nk = ks[1] - ks[0]
                    xk = xs.tile([128, KG, C], dt.float16, tag="xk")
                    nc.sync.dma_start(xk[:, :nk, :], xTr[:, ks[0]:ks[1], :])
                    wk = ws.tile([128, KG, H], dt.float16, tag="wk")
                    nc.scalar.dma_start(wk[:, :nk, :], WIr[:, ks[0]:ks[1], H:2 * H])
                    ab_tiles.append((xk, wk, ks))

                def filler_gen(tiles, dst):
                    for xk, wk, ks in tiles:
                        for i in range(ks[1] - ks[0]):
                            k = ks[0] + i
                            for m in range(4):
                                nc.tensor.matmul(
                                    dst[:, m, :],
                                    wk[:, i, m * 128:(m + 1) * 128],
                                    xk[:, i, :],
                                    start=(k == 0), stop=(k == KV - 1),
                                )
                                yield

                # PB column c=b*T+u holds pre_b for input time u; the b-scan
                # walks tb = T-1-t downward, so no host-side reversal needed.
                abgen = filler_gen(ab_tiles, PB)
                abdone = [0]
                TOTF = KV * 4

                def pump_ab(target):
                    while abdone[0] < min(target, TOTF):
                        try:
                            next(abgen)
                        except StopIteration:
                            abdone[0] = TOTF
                            return
                        abdone[0] += 1

                # columns are t-major per core: c = t*BL + b, so each scan
                # step's psum slice is contiguous (strided psum matmul
                # outputs are unsupported)
                if 'S' in phases:
                    for t in range(T):
                        sl = slice(t * BL, (t + 1) * BL)
                        slp = slice((t - 1) * BL, t * BL)
                        for m in range(4):
                            for j in range(4):
                                rf = ench[:, j * 4:(j + 1) * 4] if t == 0 else \
                                    OUTF8[:, j, slp]
                                nc.tensor.matmul(
                                    PF[:, m, sl],
                                    whh[:, j, m * 128:(m + 1) * 128], rf,
                                    start=False, stop=(j == 3),
                                    skip_group_check=True,
                                )
                        nc.scalar.activation(OUTF[:, :, sl], PF[:, :, sl],
                                             AF.Tanh)
                        if WHH_FP8:
                            nc.vector.tensor_copy(OUTF8[:, :, sl],
                                                  OUTF[:, :, sl])
                        pump_ab((t + 1) * TOTF // 126 + 1)
                if 'B' in phases or 'S' in phases:
                    pump_ab(TOTF)

                # ========== b-scan (pass B as PE filler into PF) ==========
                b_tiles = []
                for g in range(NKG):
                    ks = (g * KG, min((g + 1) * KG, KV))
                    nk = ks[1] - ks[0]
                    dk = xs.tile([128, KG, C], dt.float16, tag="xk")
                    nc.sync.dma_start(dk[:, :nk, :], dxTr[:, ks[0]:ks[1], :])
                    wkd = ws.tile([128, KG, H], dt.float16, tag="wk")
                    nc.scalar.dma_start(wkd[:, :nk, :], WIr[:, ks[0]:ks[1], 2 * H:])
                    b_tiles.append((dk, wkd, ks))
                # final-proj weight chunks: prefetch on the ACT queue
                won_tiles = {}

                def won_dma(n):
                    won = ws.tile([128, 8, VC], dt.float16, tag="won")
                    nc.scalar.dma_start(won[:], WOr[:, :, n * VC:(n + 1) * VC])
                    won_tiles[n] = won

                if 'F' in phases:
                    won_dma(0)
                    won_dma(1)

                bgen = filler_gen(b_tiles, PF)
                bdone = [0]

                def pump_b(target):
                    while bdone[0] < min(target, TOTF):
                        try:
                            next(bgen)
                        except StopIteration:
                            bdone[0] = TOTF
                            return
                        bdone[0] += 1

                if 'S' in phases:
                    for t in range(T):
                        tb = T - 1 - t
                        sl = slice(tb * BL, (tb + 1) * BL)
                        slp = slice((tb + 1) * BL, (tb + 2) * BL)
                        for m in range(4):
                            for j in range(4):
                                rb = ench[:, 16 + j * 4:16 + (j + 1) * 4] \
                                    if t == 0 else OUTB8[:, j, slp]
                                nc.tensor.matmul(
                                    PB[:, m, sl],
                                    whh[:, j, H + m * 128:H + (m + 1) * 128], rb,
                                    start=False, stop=(j == 3),
                                    skip_group_check=True,
                                )
                        nc.scalar.activation(OUTB[:, :, sl], PB[:, :, sl],
                                             AF.Tanh)
                        if WHH_FP8:
                            nc.vector.tensor_copy(OUTB8[:, :, sl],
                                                  OUTB[:, :, sl])
                        pump_b((t + 1) * TOTF // 126 + 1)
                if 'B' in phases or 'S' in phases:
                    pump_b(TOTF)

                # ========== mix phase ==========
                # PB's pre_b is fully consumed; reuse two of its column
                # regions as the tiny h0/q psum so no new pool is needed
                # while PF (pass-B result) is still alive.

                # h0 = A1 @ [h_f; h_b] + b_attn1
                for m in (range(4) if 'M' in phases else []):
                    for k in range(8):
                        rh = OUTF[:, k, (T - 1) * BL:T * BL] if k < 4 \
                            else OUTB[:, k - 4, 0:BL]
                        nc.tensor.matmul(PB[:, 0, m * 4:(m + 1) * 4],
                                         a1[:, k, m * 128:(m + 1) * 128],
                                         rh, start=(k == 0), stop=(k == 7),
                                         skip_group_check=True)
                for m in (range(4) if 'M' in phases else []):
                    nc.scalar.activation(H0[:, m, :], PB[:, 0, m * 4:(m + 1) * 4],
                                         AF.Identity, bias=cst[:, m:m + 1])
                # q = Whh_d @ h0 + bhh_d   (whh fp8 x H0 fp16 mixed when WHH_FP8;
                # cast H0 to whh dtype via DVE if needed)
                if WHH_FP8:
                    H0q = osp.tile([128, 4, 4], dt.float8e4, tag="h0q")
                    if 'M' in phases:
                        nc.vector.tensor_copy(H0q[:], H0[:])
                else:
                    H0q = H0
                # q = Whh_d @ h0 + bhh_d, psum in PB region 1
                for m in (range(4) if 'M' in phases else []):
                    for j in range(4):
                        nc.tensor.matmul(
                            PB[:, 1, m * 4:(m + 1) * 4],
                            whh[:, j, 2 * H + m * 128:2 * H + (m + 1) * 128],
                            H0q[:, j, :], start=(j == 0), stop=(j == 3),
                            skip_group_check=True,
                        )
                for m in (range(4) if 'M' in phases else []):
                    nc.scalar.activation(Q[:, m, :], PB[:, 1, m * 4:(m + 1) * 4],
                                         AF.Identity, bias=cst[:, 8 + m:9 + m])

                # DEC tanh straight from PF (pass-B) psum; psum cols are
                # t-major so batch b is the strided read b::BL, while DEC
                # itself is written b-major for contiguous matmul stationaries
                for m in (range(4) if 'M' in phases else []):
                    for b in range(BL):
                        nc.scalar.activation(
                            DEC[:, m, b * T:(b + 1) * T], PF[:, m, b::BL],
                            AF.Tanh, bias=Q[:, m, b:b + 1],
                        )

                # PB then PF dead -> LIFO pool exits, then one mix pool
                p2_cm.__exit__(None, None, None)
                p1_cm.__exit__(None, None, None)
                pmix_cm = tc.tile_pool(name="pmix", bufs=1, space="PSUM")
                pmix = pmix_cm.__enter__()

                # ENC = W_attn2 @ [out_f; out_b] + b_attn2, ENCT transposes
                # pipelined on PE behind each m's ACT
                def enc_mms(m):
                    pe = pmix.tile([128, C], dt.float32, tag="pe2", bufs=2)
                    for k in range(8):
                        src = OUTF if k < 4 else OUTB
                        nc.tensor.matmul(
                            pe[:], a2[:, k, m * 128:(m + 1) * 128], src[:, k % 4, :],
                            start=(k == 0), stop=(k == 7),
                        )
                    return pe

                def enct_tr(m):
                    for b in range(BL):
                        ptr = pmix.tile([128, 128], dt.float16, tag="ptb", bufs=2)
                        nc.tensor.transpose(ptr[:], ENC[:, m, b * T:(b + 1) * T],
                                            idn16[:])
                        nc.vector.tensor_copy(
                            ENCT[:, b, m * 128:(m + 1) * 128], ptr[:])

                pe_tiles = {}
                for m in (range(4) if 'M' in phases else []):
                    pe_tiles[m] = enc_mms(m)
                    # psum cols t-major -> ENC written b-major via AP permute
                    nc.scalar.activation(
                        ENC[:, m, :].rearrange("p (b t) -> p b t", b=BL),
                        pe_tiles[m][:].rearrange("p (t b) -> p b t", b=BL),
                        AF.Identity, bias=cst[:, 4 + m:5 + m])
                    if m > 0:
                        enct_tr(m - 1)
                if 'M' in phases:
                    enct_tr(3)

                # ---- attention, software-pipelined over batches ----
                def scores_chain(b):
                    ps = pmix.tile([128, 128], dt.float32, tag="pf128", bufs=2)
                    for k in range(4):
                        nc.tensor.matmul(
                            ps[:], DEC[:, k, b * T:(b + 1) * T],
                            ENC[:, k, b * T:(b + 1) * T],
                            start=(k == 0), stop=(k == 3),
                        )
                    negm = osp.tile([128, 1], dt.float32, tag="negm")
                    nc.vector.reduce_max(negm[:], ps[:], axis=AX.X, negate=True)
                    prob = osp.tile([128, T], dt.float16, tag="prob")
                    rsum = osp.tile([128, 1], dt.float32, tag="rsum")
                    nc.scalar.activation(prob[:], ps[:], AF.Exp, bias=negm[:],
                                         accum_out=rsum[:])
                    rinv = osp.tile([128, 1], dt.float32, tag="rinv")
                    nc.vector.reciprocal(rinv[:], rsum[:])
                    nc.vector.tensor_scalar_mul(prob[:], prob[:], rinv[:])
                    return prob

                def ctx_chain(b, prob):
                    pwt = pmix.tile([128, 128], dt.float16, tag="ptb", bufs=2)
                    nc.tensor.transpose(pwt[:], prob[:], idn16[:])
                    wt = osp.tile([128, T], dt.float16, tag="wt")
                    nc.vector.tensor_copy(wt[:], pwt[:])
                    for m in range(4):
                        pc = pmix.tile([128, 128], dt.float32, tag="pf128", bufs=2)
                        nc.tensor.matmul(pc[:], ENCT[:, b, m * 128:(m + 1) * 128],
                                         wt[:], start=True, stop=True)
                        nc.vector.tensor_copy(CTX[:, m, b * T:(b + 1) * T], pc[:])

                probs = {}
                for b in (range(BL) if 'M' in phases else []):
                    probs[b] = scores_chain(b)
                    if b > 0:
                        ctx_chain(b - 1, probs[b - 1])
                if 'M' in phases:
                    ctx_chain(3, probs[3])
                pmix_cm.__exit__(None, None, None)

                # ========== final projection: predict[c, v] ==========
                pf_cm = tc.tile_pool(name="pf", bufs=1, space="PSUM")
                pf = pf_cm.__enter__()
                for n in (range(NV) if 'F' in phases else []):
                    won = won_tiles[n]
                    pbias = pf.tile([128, VC], dt.float32, tag="pbias", bufs=2)
                    nc.tensor.matmul(pbias[:], ones[0:1, :],
                                     bout[0:1, n * VC:(n + 1) * VC],
                                     start=True, stop=True)
                    bias_sb = osp.tile([128, VC], dt.float16, tag="bsb", bufs=2)
                    nc.scalar.activation(bias_sb[:], pbias[:], AF.Copy)
                    if n + 2 < NV:
                        won_dma(n + 2)
                    ob = osp.tile([128, BL, VC], dt.float16, tag="ob", bufs=2)
                    for b in range(BL):
                        po = pf.tile([128, VC], dt.float32, tag="po", bufs=4)
                        for k in range(8):
                            src = CTX if k < 4 else DEC
                            nc.tensor.matmul(
                                po[:], src[:, k % 4, b * T:(b + 1) * T],
                                won[:, k, :],
                                start=(k == 0), stop=(k == 7),
                            )
                        nc.vector.tensor_add(ob[:, b, :], po[:], bias_sb[:])
                    nc.sync.dma_start(outr[:, :, n * VC:(n + 1) * VC], ob[:])
                pf_cm.__exit__(None, None, None)

    nc.compile()
    return nc


def _pack(inputs):
    """Host-side packing: shared weights + per-core activation shards."""
    import ml_dtypes
    f = {k: np.asarray(v, dtype=np.float32) for k, v in inputs.items()}

    def bf(a):
        return np.ascontiguousarray(a.astype(np.float16))

    def q8(a):
        return np.ascontiguousarray(a.astype(ml_dtypes.float8_e4m3))

    qw = q8 if WHH_FP8 else bf

    WIH = np.zeros((VP, 3 * H), np.float32)
    WIH[:V, 0:H] = f["Wih_f"].T
    WIH[:V, H:2 * H] = f["Wih_b"].T
    WIH[:V, 2 * H:] = f["Wih_d"].T
    WIH[V, 0:H] = f["bih_f"] + f["bhh_f"]
    WIH[V, H:2 * H] = f["bih_b"] + f["bhh_b"]
    WIH[V, 2 * H:] = f["bih_d"]

    WHH = np.concatenate([f["Whh_f"].T, f["Whh_b"].T, f["Whh_d"].T], axis=1)
    CONST = np.concatenate(
        [f["b_attn1"].reshape(4, 128).T, f["b_attn2"].reshape(4, 128).T,
         f["bhh_d"].reshape(4, 128).T], axis=1).astype(np.float32)

    shared = {
        "WIH": bf(WIH),
        "WO": bf(f["W_out"].T),
        "WHH": qw(WHH),
        "A1": bf(f["W_attn1"].T),
        "A2": bf(f["W_attn2"].T),
        "CONST": np.ascontiguousarray(CONST),
        "BOUT": bf(f["b_out"].reshape(1, V)),
        "ONES": bf(np.ones((1, 128), np.float32)),
        "IDN16": np.ascontiguousarray(np.eye(128, dtype=np.float16)),
    }

    # activations: [VP, T, B]; per-core column c = t*BL + b (t-major), pad
    # to VP with ones row at V
    def actT(a):  # [B, T, V] -> [VP, T, B]
        r = np.zeros((VP, T, B), np.float32)
        r[:V] = a.transpose(2, 1, 0)
        r[V] = 1.0
        return bf(r)

    XT = actT(f["enc_inputs"])
    DXT = actT(f["dec_inputs"])

    in_maps = []
    for core in range(NCORES):
        sl = slice(core * BL, (core + 1) * BL)
        eh = np.zeros((128, 32), np.float32)
        for d in range(2):
            hh = f["enc_hidden"][d, core * BL:(core + 1) * BL]     # [4, 512]
            eh[:, d * 16:(d + 1) * 16] = \
                hh.T.reshape(4, 128, 4).transpose(1, 0, 2).reshape(128, 16)
        m = dict(shared)
        m["xT"] = np.ascontiguousarray(XT[:, :, sl]).reshape(VP, C)
        m["dxT"] = np.ascontiguousarray(DXT[:, :, sl]).reshape(VP, C)
        m["ENCH"] = qw(eh)
        in_maps.append(m)
    return in_maps


def kernel(**inputs):
    from concourse.bass_utils import run_bass_kernel_spmd

    if "nc" not in _cached:
        _cached["nc"] = _build_nc()
    nc = _cached["nc"]
    in_maps = _pack(inputs)
    res = run_bass_kernel_spmd(
        nc, in_maps, core_ids=list(range(NCORES)),
        trace=bool(int(os.environ.get("KTRACE", "0"))),
    )
    _cached["last"] = res
    outp = np.zeros((B, T, V), np.float32)
    for core in range(NCORES):
        outp[core * BL:(core + 1) * BL] = \
            np.asarray(res.results[core]["out"], np.float32)
    return outp
